# revision 6
# baseline (speedup 1.0000x reference)
"""Distillation loss (chunked KL + CE) on 8 Trainium2 NeuronCores.

Same token-sharded scheme and host combine as the baseline (512 token
rows per core, streamed as 16 slots of [128 rows x 8000 vocab cols]),
with two performance changes:

1. fp8 (e4m3) transport: the host casts the fp32 logits to fp8
   before staging, quartering HBM->SBUF traffic.  All reductions
   accumulate in fp32 and exp(t/T) is materialized in bf16, so the
   end-to-end loss error stays ~8e-4 (tolerance 2e-2; HW-measured).
2. Loads are issued from gpsimd (SWDGE, qPoolDynamic) instead of sync
   (HWDGE, qSPDynamicHW).  SWDGE sprays a [128, w] transfer's
   descriptors across all 16 SDMA engines (HW-measured ~341 GB/s at
   1 MB); the baseline's HWDGE path measured ~22 GB/s effective on this
   part (consistent with descriptors draining through one engine).

Per-core stats produced on device (fp32 accums), identical layout to
the baseline:

    Zu  = sum exp(t/T)          acc_a[:, 3j+0]
    Zv  = sum exp(s/T)          acc_a[:, 3j+1]
    Zce = sum exp(s)            acc_a[:, 3j+2]
    W1  = sum exp(t/T) * t      acc_d[:, 2j+0]
    W2  = sum exp(t/T) * s      acc_d[:, 2j+1]

Host combine (float64):

    kl_tok_chunk = (W1 - W2) / (T * Zu) + log Zv - log Zu
    total_kl     = sum(kl) * T^2 * (chunk/V) / B
    nll_tok      = log(sum_c Zce_c) - s[label]   (gather on fp32 host copy)
    ce           = mean(nll over labels != PAD)
    loss         = ALPHA * total_kl + (1 - ALPHA) * ce

Raw Bass (manual semaphores): this container's walrus build rejects
Tile-generated instructions carrying multiple embedded sync waits
("Too many sync wait commands") and InstTensorTensorReduce entirely
("ISA wrong length"), so the kernel uses explicit engine blocks with
standalone waits and scalar_tensor_tensor for the fused dot-reductions.
"""

from contextlib import ExitStack

import ml_dtypes
import numpy as np

import concourse.bass as bass
import concourse.mybir as mybir
from concourse.bass_utils import run_bass_kernel_spmd

ALPHA = 0.7
TEMP = 5.0
PAD_ID = 0
NUM_CHUNKS = 4

N_CORES = 8
B, S, V = 2, 2048, 32000
TOK = B * S                      # 4096 tokens total
TPC = TOK // N_CORES             # 512 tokens per core
P = 128                          # SBUF partitions

W_SUB = 8000  # free-dim subtile width (= one 8000-wide chunk)


def _route_dve(j):
    """True if slot j computes Zce on DVE as exp(s/T)^5 (2 of 16 slots)."""
    return j % 8 == 1

SYNC_COLS = 1000                 # per-load column share issued by sync (of w)
IN_DT = mybir.dt.float8e4
NP_IN_DT = ml_dtypes.float8_e4m3


def _build_nc(tpc=TPC, v=V, n_chunks=NUM_CHUNKS, w=W_SUB, nbuf=3, repeat=1,
              in_dt=IN_DT, sync_cols=SYNC_COLS):
    """Per-core Bass program over shards s,t of shape [tpc, v].

    repeat>1 re-runs the whole streaming loop (for marginal-cost wall
    benchmarking); results are identical since accumulator columns are
    simply overwritten with the same values.
    """
    chw = v // n_chunks          # vocab chunk width (softmax-local)
    nsub = chw // w              # free-dim subtiles per chunk
    ntt = tpc // P               # token tiles (partition dim)
    nslot = ntt * n_chunks * nsub
    f32 = mybir.dt.float32
    EXP = mybir.ActivationFunctionType.Exp
    MULT = mybir.AluOpType.mult

    nc = bass.Bass()
    s = nc.dram_tensor("s", [tpc, v], in_dt, kind="ExternalInput")
    t = nc.dram_tensor("t", [tpc, v], in_dt, kind="ExternalInput")
    sa = nc.dram_tensor("stats_act", [P, 3 * nslot], f32, kind="ExternalOutput")
    sd = nc.dram_tensor("stats_dve", [P, 2 * nslot], f32, kind="ExternalOutput")

    # slot i covers token rows [tt*P, tt*P+P) x vocab cols [c0, c0+w)
    slots = []
    for tt in range(ntt):
        for ch in range(n_chunks):
            for sub in range(nsub):
                slots.append((tt * P, ch * chw + sub * w))
    slots = slots * repeat
    ntotal = len(slots)

    with ExitStack() as ctx:
        t_bufs = [ctx.enter_context(nc.sbuf_tensor(f"t_buf{k}", [P, w], in_dt))
                  for k in range(nbuf)]
        s_bufs = [ctx.enter_context(nc.sbuf_tensor(f"s_buf{k}", [P, w], in_dt))
                  for k in range(nbuf)]
        e_bufs = [ctx.enter_context(
            nc.sbuf_tensor(f"e_buf{k}", [P, w], mybir.dt.bfloat16))
                  for k in range(nbuf)]
        es_bufs = [ctx.enter_context(
            nc.sbuf_tensor(f"es_buf{k}", [P, w], mybir.dt.bfloat16))
                   for k in range(nbuf)]
        d_buf = ctx.enter_context(
            nc.sbuf_tensor("d_buf", [P, w], mybir.dt.bfloat16))
        es2_buf = ctx.enter_context(
            nc.sbuf_tensor("es2_buf", [P, w], mybir.dt.bfloat16))
        # Per-op discard targets for unneeded elementwise outputs.  Instead
        # of a [P,1] column broadcast to [P,w] (stride-0: every element hits
        # the same SBUF word, a potential write-port hazard on HW), each op
        # writes a small [P, sinkw] stride-1 region repeated w//sinkw times
        # (3D AP with a stride-0 middle dim).  Regions rotate with the
        # buffer parity; reuse is ordered through the et/dve sem chain.
        sinkw = next(d for d in range(128, 0, -1) if w % d == 0)
        nrep = w // sinkw
        sink_a = ctx.enter_context(
            nc.sbuf_tensor("sink_a", [P, 2 * nbuf * sinkw], f32))
        sink_d = ctx.enter_context(
            nc.sbuf_tensor("sink_d", [P, 2 * nbuf * sinkw], f32))

        def sink(tens, idx):
            base = tens[:, idx * sinkw:(idx + 1) * sinkw]
            return bass.AP(tensor=base.tensor, offset=base.offset,
                           ap=[base.ap[0], [0, nrep], [1, sinkw]])
        acc_a = ctx.enter_context(nc.sbuf_tensor("acc_a", [P, 3 * nslot], f32))
        acc_d = ctx.enter_context(nc.sbuf_tensor("acc_d", [P, 2 * nslot], f32))
        # One DMA sem per buffer parity: slot i's loads inc dma_sems[i%nbuf].
        # Reuse of a parity is gated on act/dve sems, so when ACT waits on
        # dma_sems[b] no future increments of it can be in flight -- the
        # threshold is then an exact "slot landed" signal.
        dma_sems_g = [ctx.enter_context(nc.semaphore(f"dma_g{k}"))
                      for k in range(nbuf)]
        dma_sems_h = [ctx.enter_context(nc.semaphore(f"dma_h{k}"))
                      for k in range(nbuf)]
        out_sem = ctx.enter_context(nc.semaphore("out_sem"))
        act_sem = ctx.enter_context(nc.semaphore("act_sem"))  # +1 per done slot
        et_sem = ctx.enter_context(nc.semaphore("et_sem"))    # +1 when et ready
        dve_sem = ctx.enter_context(nc.semaphore("dve_sem"))  # +1 per done slot
        ch_sem = ctx.enter_context(nc.semaphore("ch_sem"))    # DVE intra-slot chain
        block = ctx.enter_context(nc.Block())

        wg = w - sync_cols           # gpsimd column share per load

        def issue_loads(eng, lo, hi, sems):
            # eng streams cols [lo, hi) of every slot's t and s tiles;
            # both issuers follow the same parity gating, so each sem
            # reaching 32*(cycle) remains an exact "slot landed" signal.
            # SWDGE and HWDGE must not share a semaphore, hence one sem
            # array per issuing engine.
            for i, (r0, c0) in enumerate(slots):
                b = i % nbuf
                if i >= nbuf:
                    # buffers b still read by ACT/DVE of slot i-nbuf
                    eng.wait_ge(act_sem, i - nbuf + 1)
                    eng.wait_ge(dve_sem, i - nbuf + 1)
                eng.dma_start(
                    out=t_bufs[b][:, lo:hi], in_=t[r0:r0 + P, c0 + lo:c0 + hi]
                ).then_inc(sems[b], 16)
                eng.dma_start(
                    out=s_bufs[b][:, lo:hi], in_=s[r0:r0 + P, c0 + lo:c0 + hi]
                ).then_inc(sems[b], 16)

        @block.sync
        def _(sync):
            if sync_cols:
                issue_loads(sync, wg, w, dma_sems_h)
            sync.wait_ge(act_sem, ntotal)
            sync.wait_ge(dve_sem, ntotal)
            sync.dma_start(out=sa[:, :], in_=acc_a[:]).then_inc(out_sem, 16)
            sync.dma_start(out=sd[:, :], in_=acc_d[:]).then_inc(out_sem, 16)
            sync.wait_ge(out_sem, 32)

        @block.gpsimd
        def _(gpsimd):
            issue_loads(gpsimd, 0, wg, dma_sems_g)

        @block.scalar
        def _(scalar):
            scalar.wait_ge(ch_sem, 2)   # acc_a zeroed before first accum
            for i in range(ntotal):
                b = i % nbuf
                j = i % nslot  # accumulator column (repeats overwrite)
                scalar.wait_ge(dma_sems_g[b], 32 * (i // nbuf + 1))  # landed
                scalar.wait_ge(dma_sems_h[b], 32 * (i // nbuf + 1))
                if i >= nbuf:
                    scalar.wait_ge(dve_sem, i - nbuf + 1)  # e_bufs[b] free
                # et = exp(t/T); Zu partial
                nc.scalar.activation(
                    e_bufs[b][:], t_bufs[b][:], EXP, bias=0.0, scale=1.0 / TEMP,
                    accum_out=acc_a[:, 3 * j:3 * j + 1],
                ).then_inc(et_sem, 1)
                if not _route_dve(j):
                    # Zce partial = sum exp(s)
                    nc.scalar.activation(
                        sink(sink_a, 2 * b),
                        s_bufs[b][:], EXP, bias=0.0, scale=1.0,
                        accum_out=acc_a[:, 3 * j + 2:3 * j + 3],
                    )
                # es = exp(s/T); Zv partial; last ACT read of t/s this slot
                nc.scalar.activation(
                    es_bufs[b][:],
                    s_bufs[b][:], EXP, bias=0.0, scale=1.0 / TEMP,
                    accum_out=acc_a[:, 3 * j + 1:3 * j + 2],
                ).then_inc(act_sem, 1)

        @block.vector
        def _(vector):
            # ch_sem orders same-engine RAW/WAR hops (the race detector does
            # not credit engine program order for pipelined execution); a
            # slot's last op completing (dve_sem) transitively implies all
            # its chain ops completed, so the slot-top dve_sem wait also
            # covers reuse of the shared d_buf/es2_buf scratch.
            SUB = mybir.AluOpType.subtract
            # Zero both accumulators once: routed slots never write their
            # unused Zce column and the final store reads the whole tensor.
            nc.vector.memset(acc_a[:], 0).then_inc(ch_sem, 1)
            nc.vector.memset(acc_d[:], 0).then_inc(ch_sem, 1)
            ch = 2
            for i in range(ntotal):
                b = i % nbuf
                j = i % nslot
                if i > 0:
                    vector.wait_ge(dve_sem, i)   # prior slot fully done
                vector.wait_ge(et_sem, i + 1)  # et ready (implies DMA done)
                # d = t - s (bf16, 2x mode)
                nc.vector.tensor_tensor(
                    d_buf[:], t_bufs[b][:], s_bufs[b][:], SUB,
                ).then_inc(ch_sem, 1)
                ch += 1
                vector.wait_ge(ch_sem, ch)     # d fully written
                # W12 partial = sum et*(t-s)
                w12 = nc.vector.scalar_tensor_tensor(
                    out=sink(sink_d, 2 * b),
                    in0=e_bufs[b][:], scalar=1.0, in1=d_buf[:],
                    op0=MULT, op1=MULT,
                    accum_out=acc_d[:, 2 * j:2 * j + 1],
                )
                if not _route_dve(j):
                    w12.then_inc(dve_sem, 1)
                    continue
                w12.then_inc(ch_sem, 1)
                ch += 1
                vector.wait_ge(act_sem, i + 1)  # es ready
                vector.wait_ge(ch_sem, ch)      # d_buf free for es4 reuse
                # es2 = es^2
                nc.vector.tensor_tensor(
                    es2_buf[:], es_bufs[b][:], es_bufs[b][:], MULT,
                ).then_inc(ch_sem, 1)
                ch += 1
                vector.wait_ge(ch_sem, ch)
                # es4 = es2^2 (reuses d_buf)
                nc.vector.tensor_tensor(
                    d_buf[:], es2_buf[:], es2_buf[:], MULT,
                ).then_inc(ch_sem, 1)
                ch += 1
                vector.wait_ge(ch_sem, ch)
                # Zce partial = sum es4 * es = sum exp(s)
                nc.vector.scalar_tensor_tensor(
                    out=sink(sink_d, 2 * b + 1),
                    in0=d_buf[:], scalar=1.0, in1=es_bufs[b][:],
                    op0=MULT, op1=MULT,
                    accum_out=acc_d[:, 2 * j + 1:2 * j + 2],
                ).then_inc(dve_sem, 1)

    return nc


_NC_CACHE = {}
last_results = None  # BassKernelResults of the most recent run (for profiling)


def _get_nc():
    if "nc" not in _NC_CACHE:
        _NC_CACHE["nc"] = _build_nc()
    return _NC_CACHE["nc"]


def _combine(results, s_full, lab, tpc=TPC, v=V, n_chunks=NUM_CHUNKS, w=W_SUB):
    """Host-side float64 reduction of per-core partials -> scalar loss."""
    chw = v // n_chunks
    nsub = chw // w
    ntt = tpc // P
    tok = len(results) * tpc

    # [tok, n_chunks, nsub, stat] with token index = core*tpc + tt*P + p
    act = np.concatenate([
        r["stats_act"].astype(np.float64)
        .reshape(P, ntt, n_chunks, nsub, 3).transpose(1, 0, 2, 3, 4)
        .reshape(tpc, n_chunks, nsub, 3)
        for r in results], axis=0)
    dve = np.concatenate([
        r["stats_dve"].astype(np.float64)
        .reshape(P, ntt, n_chunks, nsub, 2).transpose(1, 0, 2, 3, 4)
        .reshape(tpc, n_chunks, nsub, 2)
        for r in results], axis=0)

    zu = act[..., 0].sum(axis=2)       # [tok, n_chunks]
    zv = act[..., 1].sum(axis=2)
    w12 = dve[..., 0].sum(axis=2)

    # Zce source depends on per-slot routing: slot j = (tt, ch, sub)
    nslot = ntt * n_chunks * nsub
    sel = np.empty((ntt, n_chunks, nsub), dtype=bool)
    for tt_i in range(ntt):
        for ch_i in range(n_chunks):
            for sub_i in range(nsub):
                j = (tt_i * n_chunks + ch_i) * nsub + sub_i
                sel[tt_i, ch_i, sub_i] = _route_dve(j)
    sel_tok = np.repeat(sel, P, axis=0)               # [tpc, n_chunks, nsub]
    sel_tok = np.tile(sel_tok, (len(results), 1, 1))  # [tok, ...]
    zce = np.where(sel_tok, dve[..., 1], act[..., 2]).sum(axis=(1, 2))

    kl = w12 / (TEMP * zu) + np.log(zv) - np.log(zu)
    total_kl = kl.sum() * (TEMP * TEMP) * (chw / v) / B

    s_label = s_full[np.arange(tok), lab].astype(np.float64)
    nll = np.log(zce) - s_label
    valid = lab != PAD_ID
    n_valid = max(int(valid.sum()), 1)
    ce = float(nll[valid].sum()) / n_valid

    return ALPHA * total_kl + (1.0 - ALPHA) * ce


def kernel(student_logits, teacher_logits, labels):
    global last_results
    s_full = np.ascontiguousarray(
        np.asarray(student_logits, dtype=np.float32)).reshape(TOK, V)
    t_full = np.ascontiguousarray(
        np.asarray(teacher_logits, dtype=np.float32)).reshape(TOK, V)
    lab = np.asarray(labels).reshape(TOK).astype(np.int64)

    s_lp = s_full.astype(NP_IN_DT) if IN_DT != mybir.dt.float32 else s_full
    t_lp = t_full.astype(NP_IN_DT) if IN_DT != mybir.dt.float32 else t_full

    nc = _get_nc()
    in_maps = [
        {"s": s_lp[c * TPC:(c + 1) * TPC], "t": t_lp[c * TPC:(c + 1) * TPC]}
        for c in range(N_CORES)
    ]
    last_results = run_bass_kernel_spmd(nc, in_maps, core_ids=list(range(N_CORES)))
    loss = _combine(last_results.results, s_full, lab)
    return np.array(loss, dtype=np.float32)
